# revision 16
# baseline (speedup 1.0000x reference)
"""Equivariant layer block kernel for Trainium2 (8 NeuronCores).

Math: X has shape (A=512, B=512, C=1024) with axes (a, b, c); output
Y (C, B) over (c, d).  The 10 partition terms collapse to:

  Y[c,d] = w2*P_b[d,c] + w3*P_a[d,c] + w4*T[d,c]          (matrix terms)
         + w0*S_ab[c] + w1*D[c]                            (col terms)
         + w7*Q_a[d] + w8*Q_b[d] + w9*QT[d]                (row terms)
         + w5*s + w6*sD                                    (scalar terms)

  P_b[a,c] = sum_b X[a,b,c]      P_a[b,c] = sum_a X[a,b,c]
  T[a,c]   = X[a,a,c]            S_ab[c]  = sum_ab X[a,b,c]
  D[c]     = sum_a T[a,c]        Q_a[a]   = sum_bc X;  Q_b[b] = sum_ac X
  QT[a]    = sum_c T[a,c]        s = sum X;  sD = sum_ac T

Sharding: c (dim 2, 1024) split across 8 cores -> 128 c's per core.
Everything is core-local except the row/scalar terms (pool over c),
which go through a tiny AllReduce.

The per-core shard is passed relaid as x2[a, c, b] (host transpose,
part of sharding prep) so that:
  - P_b = reduce over b is a contiguous innermost DVE reduce;
  - P_a = ones.T @ x2-tile contracts a on partitions with a single-dim
    N=512 moving operand, and each PSUM row [1, 512] is directly a row
    of P_a^T[c, b] -- the exact final layout, no transpose needed.
Matmuls run as float32r (same bits; 1 cycle/col vs 4 for fp32).
The diagonal blocks X[k*128:+128, k*128:+128, :] are passed as xd and
T is DMA-gathered from them on-device (512B contiguous runs).
Big loads alternate between the SP and ACT HWDGE rings to overlap
per-transfer overheads on one ring.
"""

import sys

sys.path.insert(0, "/opt/trn_rl_repo")

import numpy as np

import concourse.bass as bass
import concourse.bacc as bacc
import concourse.tile as tile
from concourse import mybir
from concourse.bass_utils import run_bass_kernel_spmd

F32 = mybir.dt.float32
F32R = mybir.dt.float32r

A = 512  # axis a (input dim 0)
B = 512  # axis b (input dim 1)
C = 1024  # axis c (input dim 2, sharded)
CS = C // 8  # per-core c shard = 128
NAC = 4  # a chunks of 128
NCB = 16  # c blocks per core
CSUB = CS // NCB  # c's per block = 8

_CACHE = {}


def _build() -> bass.Bass:
    nc = bacc.Bacc("TRN2", num_devices=8)
    x2 = nc.dram_tensor("x2", [A, CS, B], F32R, kind="ExternalInput")
    xd = nc.dram_tensor("xd", [NAC, 128, 128, CS], F32, kind="ExternalInput")
    w = nc.dram_tensor("w", [1, 16], F32, kind="ExternalInput")
    y = nc.dram_tensor("y", [CS, B], F32, kind="ExternalOutput")
    eye_d = nc.inline_tensor(np.eye(128, dtype=np.float32), "eye_const")
    cc_in = nc.dram_tensor("cc_in", [1, 1032], F32)
    cc_out = nc.dram_tensor("cc_out", [8, 1032], F32, addr_space="Shared")

    with tile.TileContext(nc) as tc:
        with (
            tc.tile_pool(name="persist", bufs=1) as pp,
            tc.tile_pool(name="xp", bufs=8) as xp,
            tc.tile_pool(name="rp", bufs=4) as rp,
        ):
            # ---- constants / weights ----
            ones_col = pp.tile([128, 1], F32)  # ones on 128 partitions
            nc.gpsimd.memset(ones_col[:], 1.0)
            ones_row = pp.tile([1, 512], F32)  # ones on partition 0
            nc.gpsimd.memset(ones_row[:], 1.0)
            eye_sb = pp.tile([128, 128], F32)
            nc.sync.dma_start(eye_sb[:], eye_d[:])
            wrow = pp.tile([1, 16], F32)
            nc.sync.dma_start(wrow[:], w[:])
            w_sb = pp.tile([128, 16], F32)

            # ---- persistent accumulators ----
            paT2 = pp.tile([128, B], F32)  # P_a^T: [c, b]
            pbAcc = pp.tile([128, NAC, 128], F32)  # P_b: [a', (ac, c)], a = ac*128+a'
            tsb = pp.tile([128, NAC, 128], F32)  # T:   [a', (ac, c)]

            # ---- diagonal T from the xd blocks (512B contiguous runs) ----
            xdf = xd[:].rearrange("k a b c -> (k a b) c")
            for ac in range(NAC):
                st = ac * 128 * 128
                dg = xdf[st : st + 127 * 129 + 1 : 129]
                nc.gpsimd.dma_start(tsb[:, ac, :], dg)

            with tc.tile_pool(name="ps0", bufs=1, space="PSUM") as ps0:
                psw = ps0.tile([128, 16], F32)
                nc.tensor.matmul(
                    psw[:], ones_row[0:1, 0:128], wrow[:], start=True, stop=True
                )
                nc.vector.tensor_copy(w_sb[:], psw[:])

            # ---- main streaming loop over c-blocks ----
            xv = x2[:].rearrange(
                "(ac p) (cb cs) b -> cb ac p cs b", ac=NAC, cs=CSUB
            )
            dma_engines = [nc.sync, nc.scalar]
            with tc.tile_pool(name="psa", bufs=1, space="PSUM") as psa:
                for cb in range(NCB):
                    xts = []
                    for ac in range(NAC):
                        xt = xp.tile([128, CSUB, B], F32R, tag="xt")
                        dma_engines[(cb * NAC + ac) % 2].dma_start(xt[:], xv[cb, ac])
                        xts.append(xt)
                    pgs = [
                        psa.tile([1, B], F32, tag=f"pg{ci}", name=f"pg{ci}_{cb}")
                        for ci in range(CSUB)
                    ]
                    # P_a: ones.T @ X contracts a; PSUM row = P_a^T[c, :]
                    for ac in range(NAC):
                        for ci in range(CSUB):
                            nc.tensor.matmul(
                                pgs[ci][:],
                                ones_col[:].bitcast(F32R),
                                xts[ac][:, ci, :],
                                start=(ac == 0),
                                stop=(ac == NAC - 1),
                                skip_group_check=True,
                            )
                    # evacuate rows c = cb*8+ci of P_a^T.  Compute engines
                    # only address quadrant-aligned partition bases, so go
                    # PSUM -> partition-0 scratch (ACT), then SWDGE DMA
                    # scatters to the target row.
                    for ci in range(CSUB):
                        g = cb * CSUB + ci
                        sc = rp.tile([1, B], F32, tag="evsc", name=f"sc{cb}_{ci}")
                        nc.scalar.copy(sc[:], pgs[ci][:])
                        nc.gpsimd.dma_start(paT2[g : g + 1, :], sc[:])
                    # P_b: contiguous innermost reduce over b, keeps (cs)
                    for ac in range(NAC):
                        nc.vector.reduce_sum(
                            pbAcc[:, ac, cb * CSUB : (cb + 1) * CSUB],
                            xts[ac][:].bitcast(F32),
                            axis=mybir.AxisListType.X,
                        )

            with tc.tile_pool(name="pst", bufs=1, space="PSUM") as pst:
                # ---- row-term partials (feed the AllReduce) ----
                qa = pp.tile([128, NAC], F32)
                qt = pp.tile([128, NAC], F32)
                for ac in range(NAC):
                    nc.vector.reduce_sum(
                        qa[:, ac : ac + 1], pbAcc[:, ac, :], axis=mybir.AxisListType.X
                    )
                    nc.vector.reduce_sum(
                        qt[:, ac : ac + 1], tsb[:, ac, :], axis=mybir.AxisListType.X
                    )
                rq = pp.tile([128, 4], F32)
                rtmp = pp.tile([128, 4], F32)
                nc.vector.tensor_scalar_mul(rq[:], qa[:], w_sb[:, 7:8])
                nc.vector.tensor_scalar_mul(rtmp[:], qt[:], w_sb[:, 9:10])
                nc.vector.tensor_add(rq[:], rq[:], rtmp[:])
                # [128, 4] -> [4, 128] so d = col*128 + part flattens row-major
                psT = pst.tile([4, 128], F32)
                nc.tensor.matmul(psT[:], rq[:], eye_sb[:], is_transpose=True)
                rqT = pp.tile([4, 128], F32)
                nc.vector.tensor_copy(rqT[:], psT[:])
                # Q_b[b] = sum_c P_a^T[c, b]: one partition-reduce matmul
                psQb = pst.tile([1, B], F32)
                nc.tensor.matmul(
                    psQb[:], ones_col[:], paT2[:], start=True, stop=True
                )
                qbw = pp.tile([1, B], F32)
                nc.vector.tensor_scalar_mul(qbw[:], psQb[:], w_sb[0:1, 8:9])

                # ---- col terms S_ab, D and scalar partials ----
                psS = pst.tile([1, 128], F32)
                psD = pst.tile([1, 128], F32)
                for ac in range(NAC):
                    nc.tensor.matmul(
                        psS[:],
                        ones_col[:],
                        pbAcc[:, ac, :],
                        start=(ac == 0),
                        stop=(ac == NAC - 1),
                    )
                    nc.tensor.matmul(
                        psD[:],
                        ones_col[:],
                        tsb[:, ac, :],
                        start=(ac == 0),
                        stop=(ac == NAC - 1),
                    )
                sS = pp.tile([1, 128], F32)
                sD = pp.tile([1, 128], F32)
                nc.vector.tensor_copy(sS[:], psS[:])
                nc.vector.tensor_copy(sD[:], psD[:])
                colrow = pp.tile([1, 128], F32)
                ctmp = pp.tile([1, 128], F32)
                nc.vector.tensor_scalar_mul(colrow[:], sS[:], w_sb[0:1, 0:1])
                nc.vector.tensor_scalar_mul(ctmp[:], sD[:], w_sb[0:1, 1:2])
                nc.vector.tensor_add(colrow[:], colrow[:], ctmp[:])
                red2 = pp.tile([1, 2], F32)
                nc.vector.reduce_sum(red2[0:1, 0:1], sS[:], axis=mybir.AxisListType.X)
                nc.vector.reduce_sum(red2[0:1, 1:2], sD[:], axis=mybir.AxisListType.X)
                scpad = pp.tile([1, 8], F32)
                nc.vector.memset(scpad[:], 0.0)
                tmp2 = pp.tile([1, 2], F32)
                nc.vector.tensor_scalar_mul(
                    tmp2[0:1, 0:1], red2[0:1, 0:1], w_sb[0:1, 5:6]
                )
                nc.vector.tensor_scalar_mul(
                    tmp2[0:1, 1:2], red2[0:1, 1:2], w_sb[0:1, 6:7]
                )
                nc.vector.tensor_add(
                    scpad[0:1, 0:1], tmp2[0:1, 0:1], tmp2[0:1, 1:2]
                )

                # ---- AllReduce payload: w7*Qa+w9*QT | w8*Qb | scalar|pad ----
                nc.sync.dma_start(
                    cc_in[0:1, 0:512].rearrange("r (p f) -> (r p) f", p=4), rqT[:]
                )
                nc.sync.dma_start(cc_in[0:1, 512:1024], qbw[:])
                nc.sync.dma_start(cc_in[0:1, 1024:1032], scpad[:])
                nc.gpsimd.collective_compute(
                    "AllGather",
                    mybir.AluOpType.bypass,
                    replica_groups=[list(range(8))],
                    ins=[cc_in[:]],
                    outs=[cc_out[:]],
                )
                rg8 = pp.tile([8, 1032], F32)
                nc.sync.dma_start(rg8[:], cc_out[:])
                ones8 = pp.tile([8, 1], F32)
                nc.gpsimd.memset(ones8[:], 1.0)
                psg1 = pst.tile([1, 512], F32, tag="psQb")
                psg3 = pst.tile([1, 8], F32, tag="psD")
                nc.tensor.matmul(
                    psg1[:], ones8[:], rg8[:, 0:512], start=True, stop=False,
                    skip_group_check=True,
                )
                nc.tensor.matmul(
                    psg1[:], ones8[:], rg8[:, 512:1024], start=False, stop=True,
                    skip_group_check=True,
                )
                nc.tensor.matmul(
                    psg3[:], ones8[:], rg8[:, 1024:1032], start=True, stop=True
                )
                row2 = pp.tile([1, 512], F32)
                nc.vector.tensor_scalar_add(row2[:], psg1[:], psg3[0:1, 0:1])

                # ---- assemble Y ----
                # PE transpose is a pure permutation move; transpose unscaled
                # into fresh PSUM tiles, weighted-combine on ACT/DVE.  P_a^T
                # is already in final layout.
                ysb = pp.tile([128, 512], F32)
                for ac in range(NAC):
                    psB = pst.tile([128, 128], F32, tag="psB", name=f"psB{ac}")
                    psT2 = pst.tile([128, 128], F32, tag="psT2", name=f"psT2{ac}")
                    nc.tensor.matmul(
                        psB[:], pbAcc[:, ac, :], eye_sb[:], is_transpose=True,
                        start=True, stop=True,
                    )
                    nc.tensor.matmul(
                        psT2[:], tsb[:, ac, :], eye_sb[:], is_transpose=True,
                        start=True, stop=True,
                    )
                    q = ysb[:, ac * 128 : (ac + 1) * 128]
                    tq1 = rp.tile([128, 128], F32, tag="tq1", name=f"tq1_{ac}")
                    tq2 = rp.tile([128, 128], F32, tag="tq2", name=f"tq2_{ac}")
                    tq3 = rp.tile([128, 128], F32, tag="tq3", name=f"tq3_{ac}")
                    nc.scalar.mul(tq1[:], psB[:], w_sb[:, 2:3])
                    nc.vector.tensor_scalar_mul(
                        tq2[:], paT2[:, ac * 128 : (ac + 1) * 128], w_sb[:, 3:4]
                    )
                    nc.scalar.mul(tq3[:], psT2[:], w_sb[:, 4:5])
                    nc.vector.tensor_add(q, tq1[:], tq2[:])
                    nc.vector.tensor_add(q, q, tq3[:])
                # colvec: [1,128] -> [128,1] via 1x1-permutation transpose
                psCV = pst.tile([128, 1], F32)
                nc.tensor.matmul(
                    psCV[:], colrow[:], ones_col[0:1, 0:1], is_transpose=True,
                    start=True, stop=True,
                )
                colv = pp.tile([128, 1], F32)
                nc.vector.tensor_copy(colv[:], psCV[:])
                nc.scalar.add(ysb[:], ysb[:], colv[:, 0:1])
                # row terms + global scalar: broadcast row2 over partitions
                psRow = pst.tile([128, 512], F32)
                nc.tensor.matmul(
                    psRow[:], ones_row[0:1, 0:128], row2[:], start=True, stop=True
                )
                nc.vector.tensor_add(ysb[:], ysb[:], psRow[:])
                nc.sync.dma_start(y[:], ysb[:])
    nc.compile()
    return nc


def _get_nc() -> bass.Bass:
    if "nc" not in _CACHE:
        _CACHE["nc"] = _build()
    return _CACHE["nc"]


def _run(X: np.ndarray, w: np.ndarray, **kwargs):
    nc = _get_nc()
    wpad = np.zeros((1, 16), dtype=np.float32)
    wpad[0, :10] = np.asarray(w, dtype=np.float32).reshape(-1)
    X = np.asarray(X, dtype=np.float32)
    XT = np.ascontiguousarray(X.transpose(0, 2, 1))  # (a, c, b)
    xd_full = np.stack(
        [X[k * 128 : (k + 1) * 128, k * 128 : (k + 1) * 128, :] for k in range(NAC)]
    )
    in_maps = []
    for k in range(8):
        x2 = np.ascontiguousarray(XT[:, k * CS : (k + 1) * CS, :])
        xdk = np.ascontiguousarray(xd_full[:, :, :, k * CS : (k + 1) * CS])
        in_maps.append({"x2": x2, "xd": xdk, "w": wpad})
    res = run_bass_kernel_spmd(nc, in_maps, core_ids=list(range(8)), **kwargs)
    Y = np.concatenate([r["y"] for r in res.results], axis=0)
    return Y, res


def kernel(X: np.ndarray, weights: np.ndarray) -> np.ndarray:
    X = np.asarray(X, dtype=np.float32)
    Y, _ = _run(X, weights)
    return Y


# revision 18
# speedup vs baseline: 1.0370x; 1.0370x over previous
"""Equivariant layer block kernel for Trainium2 (8 NeuronCores).

Math: X has shape (A=512, B=512, C=1024) with axes (a, b, c); output
Y (C, B) over (c, d).  The 10 partition terms collapse to:

  Y[c,d] = w2*P_b[d,c] + w3*P_a[d,c] + w4*T[d,c]          (matrix terms)
         + w0*S_ab[c] + w1*D[c]                            (col terms)
         + w7*Q_a[d] + w8*Q_b[d] + w9*QT[d]                (row terms)
         + w5*s + w6*sD                                    (scalar terms)

  P_b[a,c] = sum_b X[a,b,c]      P_a[b,c] = sum_a X[a,b,c]
  T[a,c]   = X[a,a,c]            S_ab[c]  = sum_ab X[a,b,c]
  D[c]     = sum_a T[a,c]        Q_a[a]   = sum_bc X;  Q_b[b] = sum_ac X
  QT[a]    = sum_c T[a,c]        s = sum X;  sD = sum_ac T

Sharding: c (dim 2, 1024) split across 8 cores -> 128 c's per core.
Everything is core-local except the row/scalar terms (pool over c),
which go through a tiny AllReduce.

The per-core shard is passed relaid as x2[a, c, b] (host transpose,
part of sharding prep) so that:
  - P_b = reduce over b is a contiguous innermost DVE reduce;
  - P_a = ones.T @ x2-tile contracts a on partitions with a single-dim
    N=512 moving operand, and each PSUM row [1, 512] is directly a row
    of P_a^T[c, b] -- the exact final layout, no transpose needed.
Matmuls run as float32r (same bits; 1 cycle/col vs 4 for fp32).
The diagonal blocks X[k*128:+128, k*128:+128, :] are passed as xd and
T is DMA-gathered from them on-device (512B contiguous runs).
Big loads alternate between the SP and ACT HWDGE rings to overlap
per-transfer overheads on one ring.
"""

import sys

sys.path.insert(0, "/opt/trn_rl_repo")

import numpy as np

import concourse.bass as bass
import concourse.bacc as bacc
import concourse.tile as tile
from concourse import mybir
from concourse.bass_utils import run_bass_kernel_spmd

F32 = mybir.dt.float32
F32R = mybir.dt.float32r

A = 512  # axis a (input dim 0)
B = 512  # axis b (input dim 1)
C = 1024  # axis c (input dim 2, sharded)
CS = C // 8  # per-core c shard = 128
NAC = 4  # a chunks of 128
NCB = 16  # c blocks per core
CSUB = CS // NCB  # c's per block = 8

_CACHE = {}


def _build() -> bass.Bass:
    nc = bacc.Bacc("TRN2", num_devices=8)
    x2 = nc.dram_tensor("x2", [A, CS, B], F32R, kind="ExternalInput")
    xd = nc.dram_tensor("xd", [NAC, 128, 128, CS], F32, kind="ExternalInput")
    w = nc.dram_tensor("w", [1, 16], F32, kind="ExternalInput")
    y = nc.dram_tensor("y", [CS, B], F32, kind="ExternalOutput")
    eye_d = nc.inline_tensor(np.eye(128, dtype=np.float32), "eye_const")
    cc_in = nc.dram_tensor("cc_in", [1, 1032], F32)
    cc_out = nc.dram_tensor("cc_out", [1, 1032], F32, addr_space="Shared")

    with tile.TileContext(nc) as tc:
        with (
            tc.tile_pool(name="persist", bufs=1) as pp,
            tc.tile_pool(name="xp", bufs=8) as xp,
            tc.tile_pool(name="rp", bufs=4) as rp,
        ):
            # ---- constants / weights ----
            ones_col = pp.tile([128, 1], F32)  # ones on 128 partitions
            nc.gpsimd.memset(ones_col[:], 1.0)
            ones_row = pp.tile([1, 512], F32)  # ones on partition 0
            nc.gpsimd.memset(ones_row[:], 1.0)
            eye_sb = pp.tile([128, 128], F32)
            nc.sync.dma_start(eye_sb[:], eye_d[:])
            wrow = pp.tile([1, 16], F32)
            nc.sync.dma_start(wrow[:], w[:])
            w_sb = pp.tile([128, 16], F32)

            # ---- persistent accumulators ----
            paT2 = pp.tile([128, B], F32)  # P_a^T: [c, b]
            pbAcc = pp.tile([128, NAC, 128], F32)  # P_b: [a', (ac, c)], a = ac*128+a'
            tsb = pp.tile([128, NAC, 128], F32)  # T:   [a', (ac, c)]

            # ---- diagonal T from the xd blocks (512B contiguous runs) ----
            xdf = xd[:].rearrange("k a b c -> (k a b) c")
            for ac in range(NAC):
                st = ac * 128 * 128
                dg = xdf[st : st + 127 * 129 + 1 : 129]
                nc.gpsimd.dma_start(tsb[:, ac, :], dg)

            with tc.tile_pool(name="ps0", bufs=1, space="PSUM") as ps0:
                psw = ps0.tile([128, 16], F32)
                nc.tensor.matmul(
                    psw[:], ones_row[0:1, 0:128], wrow[:], start=True, stop=True
                )
                nc.vector.tensor_copy(w_sb[:], psw[:])

            # ---- main streaming loop over c-blocks ----
            xv = x2[:].rearrange(
                "(ac p) (cb cs) b -> cb ac p cs b", ac=NAC, cs=CSUB
            )
            dma_engines = [nc.sync, nc.scalar]
            with tc.tile_pool(name="psa", bufs=1, space="PSUM") as psa:
                for cb in range(NCB):
                    xts = []
                    for ac in range(NAC):
                        xt = xp.tile([128, CSUB, B], F32R, tag="xt")
                        dma_engines[(cb * NAC + ac) % 2].dma_start(xt[:], xv[cb, ac])
                        xts.append(xt)
                    pgs = [
                        psa.tile([1, B], F32, tag=f"pg{ci}", name=f"pg{ci}_{cb}")
                        for ci in range(CSUB)
                    ]
                    # P_a: ones.T @ X contracts a; PSUM row = P_a^T[c, :]
                    for ac in range(NAC):
                        for ci in range(CSUB):
                            nc.tensor.matmul(
                                pgs[ci][:],
                                ones_col[:].bitcast(F32R),
                                xts[ac][:, ci, :],
                                start=(ac == 0),
                                stop=(ac == NAC - 1),
                                skip_group_check=True,
                            )
                    # evacuate rows c = cb*8+ci of P_a^T.  Compute engines
                    # only address quadrant-aligned partition bases, so go
                    # PSUM -> partition-0 scratch (ACT), then SWDGE DMA
                    # scatters to the target row.
                    for ci in range(CSUB):
                        g = cb * CSUB + ci
                        sc = rp.tile([1, B], F32, tag="evsc", name=f"sc{cb}_{ci}")
                        nc.scalar.copy(sc[:], pgs[ci][:])
                        nc.gpsimd.dma_start(paT2[g : g + 1, :], sc[:])
                    # P_b: contiguous innermost reduce over b, keeps (cs)
                    for ac in range(NAC):
                        nc.vector.reduce_sum(
                            pbAcc[:, ac, cb * CSUB : (cb + 1) * CSUB],
                            xts[ac][:].bitcast(F32),
                            axis=mybir.AxisListType.X,
                        )

            with tc.tile_pool(name="pst", bufs=1, space="PSUM") as pst:
                # ---- row-term partials (feed the AllReduce) ----
                qa = pp.tile([128, NAC], F32)
                qt = pp.tile([128, NAC], F32)
                for ac in range(NAC):
                    nc.vector.reduce_sum(
                        qa[:, ac : ac + 1], pbAcc[:, ac, :], axis=mybir.AxisListType.X
                    )
                    nc.vector.reduce_sum(
                        qt[:, ac : ac + 1], tsb[:, ac, :], axis=mybir.AxisListType.X
                    )
                rq = pp.tile([128, 4], F32)
                rtmp = pp.tile([128, 4], F32)
                nc.vector.tensor_scalar_mul(rq[:], qa[:], w_sb[:, 7:8])
                nc.vector.tensor_scalar_mul(rtmp[:], qt[:], w_sb[:, 9:10])
                nc.vector.tensor_add(rq[:], rq[:], rtmp[:])
                # [128, 4] -> [4, 128] so d = col*128 + part flattens row-major
                psT = pst.tile([4, 128], F32)
                nc.tensor.matmul(psT[:], rq[:], eye_sb[:], is_transpose=True)
                rqT = pp.tile([4, 128], F32)
                nc.vector.tensor_copy(rqT[:], psT[:])
                # Q_b[b] = sum_c P_a^T[c, b]: one partition-reduce matmul
                psQb = pst.tile([1, B], F32)
                nc.tensor.matmul(
                    psQb[:], ones_col[:], paT2[:], start=True, stop=True
                )
                pay_sb = pp.tile([1, 520], F32)
                nc.vector.tensor_scalar_mul(
                    pay_sb[0:1, 0:512], psQb[:], w_sb[0:1, 8:9]
                )

                # ---- col terms S_ab, D and scalar partials ----
                psS = pst.tile([1, 128], F32)
                psD = pst.tile([1, 128], F32)
                for ac in range(NAC):
                    nc.tensor.matmul(
                        psS[:],
                        ones_col[:],
                        pbAcc[:, ac, :],
                        start=(ac == 0),
                        stop=(ac == NAC - 1),
                    )
                    nc.tensor.matmul(
                        psD[:],
                        ones_col[:],
                        tsb[:, ac, :],
                        start=(ac == 0),
                        stop=(ac == NAC - 1),
                    )
                sS = pp.tile([1, 128], F32)
                sD = pp.tile([1, 128], F32)
                nc.vector.tensor_copy(sS[:], psS[:])
                nc.vector.tensor_copy(sD[:], psD[:])
                colrow = pp.tile([1, 128], F32)
                ctmp = pp.tile([1, 128], F32)
                nc.vector.tensor_scalar_mul(colrow[:], sS[:], w_sb[0:1, 0:1])
                nc.vector.tensor_scalar_mul(ctmp[:], sD[:], w_sb[0:1, 1:2])
                nc.vector.tensor_add(colrow[:], colrow[:], ctmp[:])
                red2 = pp.tile([1, 2], F32)
                nc.vector.reduce_sum(red2[0:1, 0:1], sS[:], axis=mybir.AxisListType.X)
                nc.vector.reduce_sum(red2[0:1, 1:2], sD[:], axis=mybir.AxisListType.X)
                nc.vector.memset(pay_sb[0:1, 512:520], 0.0)
                tmp2 = pp.tile([1, 2], F32)
                nc.vector.tensor_scalar_mul(
                    tmp2[0:1, 0:1], red2[0:1, 0:1], w_sb[0:1, 5:6]
                )
                nc.vector.tensor_scalar_mul(
                    tmp2[0:1, 1:2], red2[0:1, 1:2], w_sb[0:1, 6:7]
                )
                nc.vector.tensor_add(
                    pay_sb[0:1, 512:513], tmp2[0:1, 0:1], tmp2[0:1, 1:2]
                )

                # ---- AllReduce payload: w7*Qa+w9*QT | w8*Qb | scalar|pad ----
                nc.gpsimd.dma_start(
                    cc_in[0:1, 0:512].rearrange("r (p f) -> (r p) f", p=4),
                    rqT[:],
                )
                nc.sync.dma_start(cc_in[0:1, 512:1032], pay_sb[:])
                nc.gpsimd.collective_compute(
                    "AllReduce",
                    mybir.AluOpType.add,
                    replica_groups=[list(range(8))],
                    ins=[cc_in[:]],
                    outs=[cc_out[:]],
                )
                rg = pp.tile([1, 1032], F32)
                nc.sync.dma_start(rg[:], cc_out[:])
                row2 = pp.tile([1, 512], F32)
                nc.vector.tensor_add(row2[:], rg[0:1, 0:512], rg[0:1, 512:1024])
                nc.vector.tensor_scalar_add(row2[:], row2[:], rg[0:1, 1024:1025])

                # ---- assemble Y ----
                # PE transpose is a pure permutation move; transpose unscaled
                # into fresh PSUM tiles, weighted-combine on ACT/DVE.  P_a^T
                # is already in final layout.
                ysb = pp.tile([128, 512], F32)
                for ac in range(NAC):
                    psB = pst.tile([128, 128], F32, tag="psB", name=f"psB{ac}")
                    psT2 = pst.tile([128, 128], F32, tag="psT2", name=f"psT2{ac}")
                    nc.tensor.matmul(
                        psB[:], pbAcc[:, ac, :], eye_sb[:], is_transpose=True,
                        start=True, stop=True,
                    )
                    nc.tensor.matmul(
                        psT2[:], tsb[:, ac, :], eye_sb[:], is_transpose=True,
                        start=True, stop=True,
                    )
                    q = ysb[:, ac * 128 : (ac + 1) * 128]
                    tq1 = rp.tile([128, 128], F32, tag="tq1", name=f"tq1_{ac}")
                    tq2 = rp.tile([128, 128], F32, tag="tq2", name=f"tq2_{ac}")
                    tq3 = rp.tile([128, 128], F32, tag="tq3", name=f"tq3_{ac}")
                    nc.scalar.mul(tq1[:], psB[:], w_sb[:, 2:3])
                    nc.vector.tensor_scalar_mul(
                        tq2[:], paT2[:, ac * 128 : (ac + 1) * 128], w_sb[:, 3:4]
                    )
                    nc.scalar.mul(tq3[:], psT2[:], w_sb[:, 4:5])
                    nc.vector.tensor_add(q, tq1[:], tq2[:])
                    nc.vector.tensor_add(q, q, tq3[:])
                # colvec: [1,128] -> [128,1] via 1x1-permutation transpose
                psCV = pst.tile([128, 1], F32)
                nc.tensor.matmul(
                    psCV[:], colrow[:], ones_col[0:1, 0:1], is_transpose=True,
                    start=True, stop=True,
                )
                colv = pp.tile([128, 1], F32)
                nc.vector.tensor_copy(colv[:], psCV[:])
                nc.scalar.add(ysb[:], ysb[:], colv[:, 0:1])
                # row terms + global scalar: broadcast row2 over partitions
                psRow = pst.tile([128, 512], F32)
                nc.tensor.matmul(
                    psRow[:], ones_row[0:1, 0:128], row2[:], start=True, stop=True
                )
                nc.vector.tensor_add(ysb[:], ysb[:], psRow[:])
                nc.sync.dma_start(y[:], ysb[:])
    nc.compile()
    return nc


def _get_nc() -> bass.Bass:
    if "nc" not in _CACHE:
        _CACHE["nc"] = _build()
    return _CACHE["nc"]


def _run(X: np.ndarray, w: np.ndarray, **kwargs):
    nc = _get_nc()
    wpad = np.zeros((1, 16), dtype=np.float32)
    wpad[0, :10] = np.asarray(w, dtype=np.float32).reshape(-1)
    X = np.asarray(X, dtype=np.float32)
    XT = np.ascontiguousarray(X.transpose(0, 2, 1))  # (a, c, b)
    xd_full = np.stack(
        [X[k * 128 : (k + 1) * 128, k * 128 : (k + 1) * 128, :] for k in range(NAC)]
    )
    in_maps = []
    for k in range(8):
        x2 = np.ascontiguousarray(XT[:, k * CS : (k + 1) * CS, :])
        xdk = np.ascontiguousarray(xd_full[:, :, :, k * CS : (k + 1) * CS])
        in_maps.append({"x2": x2, "xd": xdk, "w": wpad})
    res = run_bass_kernel_spmd(nc, in_maps, core_ids=list(range(8)), **kwargs)
    Y = np.concatenate([r["y"] for r in res.results], axis=0)
    return Y, res


def kernel(X: np.ndarray, weights: np.ndarray) -> np.ndarray:
    X = np.asarray(X, dtype=np.float32)
    Y, _ = _run(X, weights)
    return Y
